# revision 8
# baseline (speedup 1.0000x reference)
"""Trainium2 Bass kernel for: conv2d(16->64, 3x3, VALID) + bias -> min over
channels -> tanh(tanh()).  Input x [64,16,256,256] f32, output [64,1,254,254].

Strategy (per core, data-parallel over batch: 8 images/core):
  - The conv is computed as matmuls with the *x-patch as the stationary
    operand* and a block-Toeplitz weight matrix as the moving operand, so the
    conv output lands as [width-positions (partitions), rows*couts (free)].
    That makes the channel-min a cheap free-dim DVE reduce_min.
  - Contraction K = 17 channels x 7 rows = 119 (channel 16 is a host-added
    ones-channel that carries the bias via an extra weight row).
  - A 7-row input window yields R=5 output rows per matmul group; the 3
    kernel x-taps (dx) are 3 PSUM-accumulated matmuls whose stationary
    operands are the same SBUF window tile sliced at column offset dx.
  - min over couts: DVE tensor_reduce(min) over the innermost 64-wide axis
    of the [127, 5, 64] PSUM view -> [127 positions, 5 rows].
  - Results accumulate in an SBUF staging tile [127 j, 254 rows]; PE
    transposes 127x127 chunks; double-tanh on ScalarE; DMA out.
"""

import sys

for _p in ("/opt/trn_rl_repo", "/root/.axon_site/_ro/trn_rl_repo"):
    if _p not in sys.path:
        sys.path.insert(0, _p)

import numpy as np

B, CIN, H, W = 64, 16, 256, 256
COUT, KK = 64, 3
HO, WO = H - 2, W - 2  # 254
N_CORES = 8
B_LOC = B // N_CORES  # 8 images per core

# geometry
WIN_ROWS = 7          # input rows per window
R = WIN_ROWS - KK + 1  # 5 output rows per window
KDIM = (CIN + 1) * WIN_ROWS  # 119 contraction rows (incl. ones channel)
NDIM = R * COUT       # 320 moving free size
MJ = 127              # output width positions per j-block
N_JB = 2              # j blocks (2*127 = 254)
N_WIN = 51            # windows: row0 = 5w for w<50, 249 for w=50
WIN_COLS = MJ + KK - 1  # 129 columns per window tile

_cache = {}


def _build_wblocks(conv_weight, conv_bias):
    """wblk[dx][rho*17+ci, r*64+co] = W[co,ci,rho-r,dx]; bias on the ones-
    channel row (rho=0, ci=CIN) of dx=0.  Partition order matches the
    [B, H, C, W] host layout of x so the window DMA merges (row, chan)."""
    wblk = np.zeros((KK, KDIM, NDIM), dtype=np.float32)
    for dx in range(KK):
        for ci in range(CIN):
            for rho in range(WIN_ROWS):
                k = rho * (CIN + 1) + ci
                for r in range(R):
                    dy = rho - r
                    if 0 <= dy < KK:
                        wblk[dx, k, r * COUT:(r + 1) * COUT] = conv_weight[:, ci, dy, dx]
    k_bias = CIN  # (rho=0, ci=16)
    for r in range(R):
        wblk[0, k_bias, r * COUT:(r + 1) * COUT] = conv_bias
    return wblk


def _build_nc(reps=1):
    import concourse.bacc as bacc
    import concourse.tile as tile
    from concourse import mybir

    f32 = mybir.dt.float32
    f32r = mybir.dt.float32r

    nc = bacc.Bacc(None)
    # x_aug host layout is [B, H, C, W]: window partitions are (row, chan)
    x_aug = nc.dram_tensor("x_aug", [B_LOC, H, CIN + 1, W], f32r, kind="ExternalInput")
    wblk_d = nc.dram_tensor("wblk", [KK, KDIM, NDIM], f32r, kind="ExternalInput")
    ident_d = nc.dram_tensor("ident", [MJ, MJ], f32, kind="ExternalInput")
    y = nc.dram_tensor("y", [B_LOC, HO, WO], f32, kind="ExternalOutput")

    with tile.TileContext(nc) as tc:
        with (
            tc.tile_pool(name="consts", bufs=1) as consts,
            tc.tile_pool(name="wins", bufs=6) as wins,
            tc.tile_pool(name="stage", bufs=3) as stage,
            tc.tile_pool(name="outs", bufs=4) as outs,
            tc.tile_pool(name="cpsum", bufs=6, space="PSUM") as cpsum,
            tc.tile_pool(name="tpsum", bufs=2, space="PSUM") as tpsum,
        ):
            wblk_s = consts.tile([KDIM, KK, NDIM], f32r)
            nc.sync.dma_start(out=wblk_s[:], in_=wblk_d.rearrange("k d n -> d k n"))
            ident_s = consts.tile([MJ, MJ], f32)
            nc.sync.dma_start(out=ident_s[:], in_=ident_d[:])

            for b in [b for _ in range(reps) for b in range(B_LOC)]:
                for jb in range(N_JB):
                    j0 = jb * MJ
                    staging = stage.tile([MJ, 256], f32)
                    for w in range(N_WIN):
                        row0 = 5 * w if w < N_WIN - 1 else HO - R
                        wt = wins.tile([KDIM, WIN_COLS], f32r)
                        nc.sync.dma_start(
                            out=wt[:],
                            in_=x_aug[b, row0:row0 + WIN_ROWS, :, j0:j0 + WIN_COLS]
                            .rearrange("r c w -> (r c) w"),
                        )
                        psum = cpsum.tile([MJ, NDIM], f32)
                        for dx in range(KK):
                            nc.tensor.matmul(
                                out=psum[:],
                                lhsT=wt[:, dx:dx + MJ],
                                rhs=wblk_s[:, dx, :],
                                start=(dx == 0),
                                stop=(dx == KK - 1),
                            )
                        nc.vector.tensor_reduce(
                            out=staging[:, row0:row0 + R],
                            in_=psum.rearrange("p (r c) -> p r c", c=COUT),
                            axis=mybir.AxisListType.X,
                            op=mybir.AluOpType.min,
                        )
                    for rb in range(2):
                        r0 = rb * MJ
                        ps_t = tpsum.tile([MJ, MJ], f32)
                        nc.tensor.transpose(
                            out=ps_t[:], in_=staging[:, r0:r0 + MJ], identity=ident_s[:]
                        )
                        t1 = outs.tile([MJ, MJ], f32)
                        nc.scalar.activation(
                            out=t1[:], in_=ps_t[:],
                            func=mybir.ActivationFunctionType.Tanh,
                        )
                        t2 = outs.tile([MJ, MJ], f32)
                        nc.scalar.activation(
                            out=t2[:], in_=t1[:],
                            func=mybir.ActivationFunctionType.Tanh,
                        )
                        nc.sync.dma_start(
                            out=y[b, r0:r0 + MJ, j0:j0 + MJ], in_=t2[:]
                        )
    nc.finalize()
    return nc


def _get_compiled(reps=1):
    key = ("nc", reps)
    if key not in _cache:
        _cache[key] = _build_nc(reps)
    return _cache[key]


def kernel(x, conv_weight, conv_bias):
    from concourse.bass_utils import run_bass_kernel_spmd

    x = np.asarray(x, dtype=np.float32)
    conv_weight = np.asarray(conv_weight, dtype=np.float32)
    conv_bias = np.asarray(conv_bias, dtype=np.float32)

    x_aug = np.empty((B, H, CIN + 1, W), dtype=np.float32)
    x_aug[:, :, :CIN] = x.transpose(0, 2, 1, 3)
    x_aug[:, :, CIN] = 1.0
    wblk = _build_wblocks(conv_weight, conv_bias)
    ident = np.eye(MJ, dtype=np.float32)

    nc = _get_compiled()
    in_maps = [
        {
            "x_aug": np.ascontiguousarray(x_aug[c * B_LOC:(c + 1) * B_LOC]),
            "wblk": wblk,
            "ident": ident,
        }
        for c in range(N_CORES)
    ]
    res = run_bass_kernel_spmd(nc, in_maps, core_ids=list(range(N_CORES)))
    out = np.concatenate([res.results[c]["y"] for c in range(N_CORES)], axis=0)
    return out.reshape(B, 1, HO, WO)
